# revision 24
# baseline (speedup 1.0000x reference)
"""AttentionDTI forward pass on 8 Trainium2 NeuronCores (pure data parallel).

One batch element per core, weights replicated. All matmul operands are
16-bit (f32 PSUM accumulation): fp16 for conv/FC weights and activations,
bf16 for the attention tiles (the Activation engine runs ~1.3x slower on
fp16 than bf16, and the attention path tolerates bf16). Embedding lookup
is done host-side; weights arrive in packed DMAs issued from both the SP
and Activation HWDGE queues so transfers overlap the NEFF prologue.

The 4D additive-attention tensor h[b,p,m,c] = relu(pa + ma) is never
materialized: mean_m(h @ Wa) == mean_m(h) @ Wa, so only hp[c,p] = sum_m h
and hm[c,m] = sum_p h are accumulated on the fly. hm accumulates in PSUM
by streaming h tiles through the PE against a stationary identity; hp
comes from fused per-tile accumulators: the Scalar engine's
relu+bias+accum activation alternates 1:1 with the Vector engine's
scalar_tensor_tensor (relu via max-with-zeros + sum accum) — any DVE op
with an accum output runs at the 1x element rate, so the fused form is
optimal on both engines.

c-channels [128:160] run packed: ma rows replicated 4x vertically (via
host-replicated stationary columns, free) so each tile covers 4 peptide
positions; a 4-stacked [128,32] identity reduces them into hm1. The
peptide gate keeps its column axis in "j-major" order (jm(p) =
(p%4)*22 + p//4) end-to-end — max-pool over p is order-invariant — which
makes the packed hp1p contributions contiguous matmuls and avoids any
unpack DMAs.

Environment constraints discovered empirically (this axon terminal):
  - GPSIMD/Pool compute ops fail codegen; SWDGE DMA hangs: DMAs go
    through SP/Activation HWDGE only.
  - tensor_tensor_reduce fails walrus codegen ("ISA wrong length").
  - tensor_scalar's accum_out hijacks op1 as the reduce op (no fused
    two-op elementwise + sum) — scalar_tensor_tensor does fuse it.
  - walrus allows at most ONE semaphore wait per instruction:
    _split_excess_waits() rewrites the scheduled program.
"""
import sys

_BASS_ROOT = '/opt/trn_rl_repo'
if _BASS_ROOT not in sys.path:
    sys.path.insert(0, _BASS_ROOT)

import numpy as np

import concourse.bass as bass
import concourse.tile as tile
from concourse import mybir
from concourse.bass_utils import run_bass_kernel_spmd

F32 = mybir.dt.float32
F16 = mybir.dt.float16
BF16 = mybir.dt.bfloat16
ALU = mybir.AluOpType
AF = mybir.ActivationFunctionType
AX = mybir.AxisListType

B = 8
LP, LM, DIM, CONV = 100, 1000, 64, 40
C2, C4 = CONV * 2, CONV * 4          # 80, 160
K1, K2, K3 = 4, 6, 8
LP1, LP2, LP3 = 97, 92, 85           # peptide conv output lengths
LM1, LM2, LM3 = 997, 992, 985        # MHC conv output lengths
NP4 = 22                             # ceil(85/4) packed p-groups
JM = 4 * NP4                         # 88 j-major gate columns
MP = 992                             # LM3 padded for 4x-eligible DVE gate ops
NEG = -30000.0
SPLIT_MOD = 2                        # i % SPLIT_MOD == 0 -> scalar h tile

# ---- wboot column map (fp16 [128, 1120]): conv1 (pair-folded) + conv2 ----
PW1, MW1, PW2, MW2 = 0, 80, 160, 640
NBOOT = 1120
# ---- wc3 column map (fp16 [128, 2560]): conv3 weights ----
PW3, MW3 = 0, 1280
NC3 = 2560
# ---- wattn column map (fp16 [128, 1632]) ----
WPA_A, WPA_B = 0, 160        # [128,160], [32,160]
WMA_A, WMA_B = 320, 576      # [128,256], [32,256] (cols 128:256 = rep4 of Wma[:,128:160])
WCA_A, WCA_B = 832, 992      # Wa/LM3: [128,160], [32,160]
WM2_A, WM2_B = 1152, 1312    # Wa/LP3
ID128, IDST = 1472, 1600     # bf16 identities: [128,128], [128,32]
WCB0, WCB1 = 1632, 2144      # j-lifted Wa[128:160]/LM3: 4x[128,128], 4x[128,32]
NATTN = 2272
# ---- wfc column map (fp16 [128, 16392]) ----
W1A, W1B = 0, 2048           # [128, 2048], [32, 2048]
W2C, W3C, WOC = 4096, 12288, 16384
NFC = 16392
# ---- wsmall column map (f32 [128, 35]) ----
SB_PB1, SB_PB2, SB_PB3A, SB_PB3B = 0, 1, 2, 3
SB_MB1, SB_MB2, SB_MB3A, SB_MB3B = 4, 5, 6, 7
SB_BPA_A, SB_BPA_B = 8, 9
SB_BMA_A, SB_BMA_R4 = 10, 11
SB_BA_A, SB_BA_B = 12, 13
SB_B1, SB_B2, SB_B3, SB_BO = 14, 22, 30, 34
NSMALL = 35


def _jm(p):
    return (p % 4) * NP4 + (p // 4)


_ctr = [0]


def _split_excess_waits(nc, max_waits=1):
    n_split = 0
    for f in nc.m.functions:
        for b in f.blocks:
            insts = list(b.instructions)
            out = []
            changed = False
            for inst in insts:
                si = inst.sync_info
                waits = list(si.on_wait) if (si is not None and si.on_wait) else []
                if len(waits) > max_waits:
                    changed = True
                    n_split += 1
                    keep = max(1, max_waits)
                    head, tail = waits[:-keep], waits[-keep:]
                    for i in range(0, len(head), keep):
                        chunk = head[i:i + keep]
                        nop = mybir.InstEventSemaphore(
                            name=f"ant-wait-split-{_ctr[0]}", ins=[], outs=[])
                        _ctr[0] += 1
                        nop.engine = inst.engine
                        nop.sync_info = mybir.SyncInfo(on_wait=chunk, on_update=[])
                        nc.register_instruction(nop)
                        out.append(nop)
                    upd = list(si.on_update) if si.on_update else []
                    inst.sync_info = mybir.SyncInfo(on_wait=tail, on_update=upd)
                out.append(inst)
            if changed:
                b.instructions = out
    return n_split


def _conv_matmuls(nc, psum, wtile, x, k_taps, co_lo, co_hi, m_lo, m_hi, cout_stride):
    """Valid 1-D conv as k shifted matmuls accumulated into `psum`."""
    for k in range(k_taps):
        nc.tensor.matmul(
            psum,
            wtile[:, k * cout_stride + co_lo: k * cout_stride + co_hi],
            x[:, m_lo + k: m_hi + k],
            start=(k == 0), stop=(k == k_taps - 1))


def _build_program():
    nc = bass.Bass("TRN2", target_bir_lowering=False, debug=False)

    emb_e = nc.declare_dram_parameter("emb", [128, LP + LM + NBOOT], F16, isOutput=False)
    wsmall_e = nc.declare_dram_parameter("wsmall", [128, NSMALL], F32, isOutput=False)
    wc3_e = nc.declare_dram_parameter("wc3", [128, NC3], F16, isOutput=False)
    wattn_e = nc.declare_dram_parameter("wattn", [128, NATTN], F16, isOutput=False)
    wfc_e = nc.declare_dram_parameter("wfc", [128, NFC], F16, isOutput=False)
    out_e = nc.declare_dram_parameter("out", [2, 1], F32, isOutput=True)

    with tile.TileContext(nc) as tc:
        with tc.tile_pool(name="consts", bufs=1) as cp, \
             tc.tile_pool(name="work", bufs=1) as wp, \
             tc.tile_pool(name="hpool", bufs=8) as hpool, \
             tc.tile_pool(name="ps_hm", bufs=1, space="PSUM") as ps_hm, \
             tc.tile_pool(name="ps_work", bufs=2, space="PSUM") as ps:

            # loads: small/boot/emb from SP; conv3+attn from Activation HWDGE;
            # the big FC pack last on SP.
            emb = cp.tile([128, LP + LM + NBOOT], F16, name="emb")
            nc.sync.dma_start(out=emb, in_=emb_e[:])
            wboot = emb[:, LP + LM:LP + LM + NBOOT]
            wsmall = cp.tile([128, NSMALL], F32, name="wsmall")
            nc.scalar.dma_start(out=wsmall, in_=wsmall_e[:])
            wc3 = cp.tile([128, NC3], F16, name="wc3")
            nc.scalar.dma_start(out=wc3, in_=wc3_e[:])
            wattn = cp.tile([128, NATTN], F16, name="wattn")
            nc.sync.dma_start(out=wattn, in_=wattn_e[:])
            wfc = cp.tile([128, NFC], F16, name="wfc")
            nc.sync.dma_start(out=wfc, in_=wfc_e[:])

            pe = emb[:, 0:LP]
            me = emb[:, LP:LP + LM]

            def conv1_pair(psum, w_base, x, m_lo, m_hi):
                # taps (0,1) and (2,3) folded via the stacked-shifted embedding
                for tp in range(2):
                    nc.tensor.matmul(
                        psum,
                        wboot[0:128, w_base + tp * CONV:w_base + (tp + 1) * CONV],
                        x[:, m_lo + 2 * tp:m_hi + 2 * tp],
                        start=(tp == 0), stop=(tp == 1))
            bias = lambda col, rows=128: wsmall[0:rows, col:col + 1]

            # early zero/NEG fills on the (idle) Vector engine
            zt = wp.tile([128, LM3], BF16, name="zt")
            nc.vector.memset(zt, 0.0)
            # warm the activation table (one-time ~1.3us load) off the
            # critical path, before any DMA lands
            atl = wp.tile([1, 1], F16, name="atl")
            nc.scalar.activation(out=atl, in_=zt[0:1, 0:1], func=AF.Relu, bias=0.0)
            hp0 = wp.tile([128, JM], F32, name="hp0")
            nc.vector.memset(hp0, 0.0)
            pa1p = wp.tile([128, NP4], F32, name="pa1p")
            nc.vector.memset(pa1p, NEG)
            pc0jm = wp.tile([128, JM], F16, name="pc0jm")
            nc.vector.memset(pc0jm, 0.0)
            pc1jm = wp.tile([32, JM], F16, name="pc1jm")
            nc.vector.memset(pc1jm, 0.0)
            hm0f = wp.tile([128, MP], BF16, name="hm0f")
            nc.vector.memset(hm0f[:, LM3:MP], 0.0)
            hm1f = wp.tile([32, MP], BF16, name="hm1f")
            nc.vector.memset(hm1f[:, LM3:MP], 0.0)
            mc0 = wp.tile([128, MP], F16, name="mc0")
            nc.vector.memset(mc0[:, LM3:MP], 0.0)
            mc1 = wp.tile([32, MP], F16, name="mc1")
            nc.vector.memset(mc1[:, LM3:MP], 0.0)

            # ================= conv stacks (fp16, f32 psum) =================
            # MHC conv1: [64,1000] -> [40,997]; relu chunked so conv2 starts early
            mx1_ps = ps.tile([CONV, LM1], F32, name="mx1_ps", tag="ps")
            conv1_pair(mx1_ps[:, 0:512], MW1, me, 0, 512)
            conv1_pair(mx1_ps[:, 512:LM1], MW1, me, 512, LM1)
            # peptide conv1 fills the PE while relu1 runs
            px1_ps = ps.tile([CONV, LP1], F32, name="px1_ps", tag="ps")
            conv1_pair(px1_ps, PW1, pe, 0, LP1)
            mx1 = wp.tile([CONV, LM1], F16, name="mx1")
            nc.scalar.activation(out=mx1[:, 0:520], in_=mx1_ps[:, 0:520], func=AF.Relu, bias=bias(SB_MB1, CONV))
            nc.vector.tensor_scalar(out=mx1[:, 520:LM1], in0=mx1_ps[:, 520:LM1], scalar1=bias(SB_MB1, CONV),
                                    scalar2=0.0, op0=ALU.add, op1=ALU.max)
            px1 = wp.tile([CONV, LP1], F16, name="px1")
            nc.scalar.activation(out=px1, in_=px1_ps, func=AF.Relu, bias=bias(SB_PB1, CONV))

            # MHC conv2 -> [80, 992]
            mx2_ps = ps.tile([C2, LM2], F32, name="mx2_ps", tag="ps")
            _conv_matmuls(nc, mx2_ps[:, 0:512], wboot[0:CONV, MW2:MW2 + K2 * C2], mx1, K2, 0, C2, 0, 512, C2)
            _conv_matmuls(nc, mx2_ps[:, 512:LM2], wboot[0:CONV, MW2:MW2 + K2 * C2], mx1, K2, 0, C2, 512, LM2, C2)
            px2_ps = ps.tile([C2, LP2], F32, name="px2_ps", tag="ps")
            _conv_matmuls(nc, px2_ps, wboot[0:CONV, PW2:PW2 + K2 * C2], px1, K2, 0, C2, 0, LP2, C2)
            mx2 = wp.tile([C2, LM2], F16, name="mx2")
            nc.scalar.activation(out=mx2[:, 0:520], in_=mx2_ps[:, 0:520], func=AF.Relu, bias=bias(SB_MB2, C2))
            nc.vector.tensor_scalar(out=mx2[:, 520:LM2], in0=mx2_ps[:, 520:LM2], scalar1=bias(SB_MB2, C2),
                                    scalar2=0.0, op0=ALU.add, op1=ALU.max)
            px2 = wp.tile([C2, LP2], F16, name="px2")
            nc.scalar.activation(out=px2, in_=px2_ps, func=AF.Relu, bias=bias(SB_PB2, C2))

            # MHC conv3 -> [160, 985] as [128,985]+[32,985] (into MP-padded tiles)
            mc0_ps = ps.tile([128, LM3], F32, name="mc0_ps", tag="ps")
            _conv_matmuls(nc, mc0_ps[:, 0:512], wc3[0:C2, MW3:MW3 + K3 * C4], mx2, K3, 0, 128, 0, 512, C4)
            _conv_matmuls(nc, mc0_ps[:, 512:LM3], wc3[0:C2, MW3:MW3 + K3 * C4], mx2, K3, 0, 128, 512, LM3, C4)
            nc.scalar.activation(out=mc0[:, 0:512], in_=mc0_ps[:, 0:512], func=AF.Relu, bias=bias(SB_MB3A))
            nc.vector.tensor_scalar(out=mc0[:, 512:LM3], in0=mc0_ps[:, 512:LM3], scalar1=bias(SB_MB3A),
                                    scalar2=0.0, op0=ALU.add, op1=ALU.max)
            mc1_ps = ps.tile([32, LM3], F32, name="mc1_ps", tag="ps")
            _conv_matmuls(nc, mc1_ps[:, 0:512], wc3[0:C2, MW3:MW3 + K3 * C4], mx2, K3, 128, C4, 0, 512, C4)
            _conv_matmuls(nc, mc1_ps[:, 512:LM3], wc3[0:C2, MW3:MW3 + K3 * C4], mx2, K3, 128, C4, 512, LM3, C4)
            nc.scalar.activation(out=mc1[:, 0:512], in_=mc1_ps[:, 0:512], func=AF.Relu, bias=bias(SB_MB3B, 32))
            nc.vector.tensor_scalar(out=mc1[:, 512:LM3], in0=mc1_ps[:, 512:LM3], scalar1=bias(SB_MB3B, 32),
                                    scalar2=0.0, op0=ALU.add, op1=ALU.max)

            # peptide conv3 (tiles padded to 88 cols for the 4-strided views)
            pc0_ps = ps.tile([128, LP3], F32, name="pc0_ps", tag="ps")
            _conv_matmuls(nc, pc0_ps, wc3[0:C2, PW3:PW3 + K3 * C4], px2, K3, 0, 128, 0, LP3, C4)
            pc0 = wp.tile([128, 88], F16, name="pc0")
            nc.scalar.activation(out=pc0[:, 0:LP3], in_=pc0_ps, func=AF.Relu, bias=bias(SB_PB3A))
            pc1_ps = ps.tile([32, LP3], F32, name="pc1_ps", tag="ps")
            _conv_matmuls(nc, pc1_ps, wc3[0:C2, PW3:PW3 + K3 * C4], px2, K3, 128, C4, 0, LP3, C4)
            pc1 = wp.tile([32, 88], F16, name="pc1")
            nc.scalar.activation(out=pc1[:, 0:LP3], in_=pc1_ps, func=AF.Relu, bias=bias(SB_PB3B, 32))

            # j-major copies of pc for the gate (vector, strided reads)
            pc0_g = pc0.rearrange("c (g f) -> c g f", f=4)
            pc1_g = pc1.rearrange("c (g f) -> c g f", f=4)
            for j in range(4):
                ncol = NP4 if j == 0 else NP4 - 1
                nc.vector.tensor_scalar(out=pc0jm[:, j * NP4:j * NP4 + ncol],
                                        in0=pc0_g[:, 0:ncol, j], scalar1=0.0,
                                        scalar2=None, op0=ALU.add)
                nc.vector.tensor_scalar(out=pc1jm[:, j * NP4:j * NP4 + ncol],
                                        in0=pc1_g[:, 0:ncol, j], scalar1=0.0,
                                        scalar2=None, op0=ALU.add)

            # ================= attention projections =================
            # ma0[c,m] c in 0:128 (bf16 for the Activation-engine h producer)
            ma0_ps = ps.tile([128, LM3], F32, name="ma0_ps", tag="ps")
            for lo, hi in ((0, 512), (512, LM3)):
                nc.tensor.matmul(ma0_ps[:, lo:hi], wattn[0:128, WMA_A:WMA_A + 128], mc0[:, lo:hi], start=True, stop=False)
                nc.tensor.matmul(ma0_ps[:, lo:hi], wattn[0:32, WMA_B:WMA_B + 128], mc1[:, lo:hi], start=False, stop=True)
            ma0 = wp.tile([128, LM3], BF16, name="ma0")
            nc.scalar.activation(out=ma0, in_=ma0_ps, func=AF.Identity, bias=bias(SB_BMA_A))

            # ma1p: c in 128:160 replicated 4x vertically (stationary pre-replicated)
            ma1p_ps = ps.tile([128, LM3], F32, name="ma1p_ps", tag="ps")
            for lo, hi in ((0, 512), (512, LM3)):
                nc.tensor.matmul(ma1p_ps[:, lo:hi], wattn[0:128, WMA_A + 128:WMA_A + 256], mc0[:, lo:hi], start=True, stop=False)
                nc.tensor.matmul(ma1p_ps[:, lo:hi], wattn[0:32, WMA_B + 128:WMA_B + 256], mc1[:, lo:hi], start=False, stop=True)
            ma1p = wp.tile([128, LM3], BF16, name="ma1p")
            nc.vector.tensor_scalar(out=ma1p, in0=ma1p_ps, scalar1=bias(SB_BMA_R4), scalar2=None, op0=ALU.add)

            # pa0[c,p] c in 0:128 (f32, used as per-partition bias)
            pa0_ps = ps.tile([128, LP3], F32, name="pa0_ps", tag="ps")
            nc.tensor.matmul(pa0_ps, wattn[0:128, WPA_A:WPA_A + 128], pc0[:, 0:LP3], start=True, stop=False)
            nc.tensor.matmul(pa0_ps, wattn[0:32, WPA_B:WPA_B + 128], pc1[0:32, 0:LP3], start=False, stop=True)
            pa0 = wp.tile([128, LP3], F32, name="pa0")
            nc.scalar.add(pa0, pa0_ps, bias(SB_BPA_A))

            # pa1p[32j+d, g] = pa[128+d, 4g+j]: partition-offset matmuls over
            # 4-strided moving views of pc
            pa1p_ps = ps.tile([128, NP4], F32, name="pa1p_ps", tag="ps")
            for j in range(4):
                ncol = NP4 if j == 0 else NP4 - 1
                nc.tensor.matmul(pa1p_ps[32 * j:32 * j + 32, 0:ncol],
                                 wattn[0:128, WPA_A + 128:WPA_A + 160],
                                 pc0_g[:, 0:ncol, j],
                                 start=True, stop=False, skip_group_check=True,
                                 tile_position=(0, 32 * j))
                nc.tensor.matmul(pa1p_ps[32 * j:32 * j + 32, 0:ncol],
                                 wattn[0:32, WPA_B + 128:WPA_B + 160],
                                 pc1_g[:, 0:ncol, j],
                                 start=False, stop=True, skip_group_check=True,
                                 tile_position=(0, 32 * j))
                nc.scalar.add(pa1p[32 * j:32 * j + 32, 0:ncol],
                              pa1p_ps[32 * j:32 * j + 32, 0:ncol], bias(SB_BPA_B, 32))

            # ================= 4D attention reductions =================
            # hp0 columns are written in j-major order (gate is order-free)
            hp1p = wp.tile([128, NP4], F32, name="hp1p")
            hm0_ps = ps_hm.tile([128, LM3], F32, name="hm0_ps", tag="hma")
            hm1_ps = ps_hm.tile([32, LM3], F32, name="hm1_ps", tag="hmb")

            # identities and gate weights stored as bf16 bit patterns in the
            # fp16 pack; bitcast views pair them with bf16 moving operands
            id128 = wattn[0:128, ID128:ID128 + 128].bitcast(BF16)
            idst = wattn[0:128, IDST:IDST + 32].bitcast(BF16)

            # greedy producer balance: assign each tile to whichever engine
            # would finish it first (measured fused costs: ACT 1199, DVE 1263)
            prod_t = {'sc': 0.0, 've': 0.0}

            def h_tile(i, src, bias_ap, acc):
                h = hpool.tile([128, LM3], BF16, tag="h", name="h")
                if prod_t['sc'] + 1200.0 <= prod_t['ve'] + 1187.0:
                    prod_t['sc'] += 1200.0
                    nc.scalar.activation(out=h, in_=src, func=AF.Relu,
                                         bias=bias_ap, accum_out=acc)
                else:
                    prod_t['ve'] += 1187.0
                    nc.vector.scalar_tensor_tensor(out=h, in0=src, scalar=bias_ap,
                                                   in1=zt, op0=ALU.add, op1=ALU.max,
                                                   accum_out=acc)
                return h

            for p in range(LP3):
                h = h_tile(p, ma0, pa0[:, p:p + 1], hp0[:, _jm(p):_jm(p) + 1])
                nc.tensor.matmul(hm0_ps[:, 0:512], id128, h[:, 0:512],
                                 start=(p == 0), stop=(p == LP3 - 1))
                nc.tensor.matmul(hm0_ps[:, 512:LM3], id128, h[:, 512:LM3],
                                 start=(p == 0), stop=(p == LP3 - 1))

            # p-side gate head start: hp0 is complete before the packed loop
            hp0f = wp.tile([128, JM], BF16, name="hp0f")
            nc.scalar.copy(hp0f, hp0)
            prod_t['sc'] += 260.0
            cl0_ps = ps.tile([128, JM], F32, name="cl0_ps", tag="ps")
            nc.tensor.matmul(cl0_ps, wattn[0:128, WCA_A:WCA_A + 128].bitcast(BF16), hp0f,
                             start=True, stop=False, skip_group_check=True)
            cl1_ps = ps.tile([32, JM], F32, name="cl1_ps", tag="ps")
            nc.tensor.matmul(cl1_ps, wattn[0:128, WCA_A + 128:WCA_A + 160].bitcast(BF16), hp0f,
                             start=True, stop=False, skip_group_check=True)

            # hm0 is also complete: copy it out and accumulate ml's hm0-part
            # in psum (reusing hm0's banks via the pool tag) during the
            # packed loop; producers absorb the copies via the greedy budget
            nc.scalar.activation(out=hm0f[:, 0:512], in_=hm0_ps[:, 0:512], func=AF.Copy)
            prod_t['sc'] += 660.0
            nc.vector.tensor_scalar(out=hm0f[:, 512:LM3], in0=hm0_ps[:, 512:LM3],
                                    scalar1=0.0, scalar2=None, op0=ALU.add)
            prod_t['ve'] += 680.0
            ml0_ps = ps_hm.tile([128, MP], F32, name="ml0_ps", tag="hma")
            for lo, hi in ((0, 512), (512, MP)):
                nc.tensor.matmul(ml0_ps[:, lo:hi], wattn[0:128, WM2_A:WM2_A + 128].bitcast(BF16),
                                 hm0f[:, lo:hi], start=True, stop=False)

            for g in range(NP4):
                h = h_tile(LP3 + g, ma1p, pa1p[:, g:g + 1], hp1p[:, g:g + 1])
                nc.tensor.matmul(hm1_ps[:, 0:512], idst, h[:, 0:512],
                                 start=(g == 0), stop=(g == NP4 - 1))
                nc.tensor.matmul(hm1_ps[:, 512:LM3], idst, h[:, 512:LM3],
                                 start=(g == 0), stop=(g == NP4 - 1))

            # ================= peptide gate (j-major) =================
            hp1pf = wp.tile([128, NP4], BF16, name="hp1pf")
            nc.scalar.copy(hp1pf, hp1p)
            for j in range(4):
                nc.tensor.matmul(cl0_ps[:, j * NP4:(j + 1) * NP4],
                                 wattn[0:128, WCB0 + j * 128:WCB0 + (j + 1) * 128].bitcast(BF16),
                                 hp1pf, start=False, stop=(j == 3), skip_group_check=True)
                nc.tensor.matmul(cl1_ps[:, j * NP4:(j + 1) * NP4],
                                 wattn[0:128, WCB1 + j * 32:WCB1 + (j + 1) * 32].bitcast(BF16),
                                 hp1pf, start=False, stop=(j == 3), skip_group_check=True)
            catt0 = wp.tile([128, JM], F16, name="catt0")
            nc.scalar.activation(out=catt0, in_=cl0_ps, func=AF.Sigmoid, bias=bias(SB_BA_A))
            catt1 = wp.tile([32, JM], F16, name="catt1")
            nc.scalar.activation(out=catt1, in_=cl1_ps, func=AF.Sigmoid, bias=bias(SB_BA_B, 32))

            pvf = wp.tile([128, 4], F16, name="pvf")   # cols: pv0, pv1, mv0, mv1
            nc.vector.memset(pvf, 0.0)
            pg0 = wp.tile([128, JM], F16, name="pg0")
            nc.vector.scalar_tensor_tensor(out=pg0, in0=catt0, scalar=0.5, in1=pc0jm,
                                           op0=ALU.add, op1=ALU.mult)
            pg1 = wp.tile([32, JM], F16, name="pg1")
            nc.vector.scalar_tensor_tensor(out=pg1, in0=catt1, scalar=0.5, in1=pc1jm,
                                           op0=ALU.add, op1=ALU.mult)
            with nc.allow_low_precision(reason="fp16 max-pool rounds values only"):
                nc.vector.tensor_reduce(out=pvf[:, 0:1], in_=pg0, op=ALU.max, axis=AX.X)
                nc.vector.tensor_reduce(out=pvf[0:32, 1:2], in_=pg1, op=ALU.max, axis=AX.X)

            # ================= MHC gate (chunk-pipelined) =================
            nc.scalar.activation(out=hm1f[:, 0:512], in_=hm1_ps[:, 0:512], func=AF.Copy)
            nc.vector.tensor_scalar(out=hm1f[:, 512:LM3], in0=hm1_ps[:, 512:LM3],
                                    scalar1=0.0, scalar2=None, op0=ALU.add)
            matt0 = wp.tile([128, MP], F16, name="matt0")
            matt1 = wp.tile([32, MP], F16, name="matt1")
            ml1_ps = ps_hm.tile([32, MP], F32, name="ml1_ps", tag="hmb")
            for lo, hi in ((0, 512), (512, MP)):
                nc.tensor.matmul(ml0_ps[:, lo:hi], wattn[0:32, WM2_B:WM2_B + 128].bitcast(BF16), hm1f[:, lo:hi], start=False, stop=True)
                nc.scalar.activation(out=matt0[:, lo:hi], in_=ml0_ps[:, lo:hi], func=AF.Sigmoid, bias=bias(SB_BA_A))
                nc.tensor.matmul(ml1_ps[:, lo:hi], wattn[0:128, WM2_A + 128:WM2_A + 160].bitcast(BF16), hm0f[:, lo:hi], start=True, stop=False)
                nc.tensor.matmul(ml1_ps[:, lo:hi], wattn[0:32, WM2_B + 128:WM2_B + 160].bitcast(BF16), hm1f[:, lo:hi], start=False, stop=True)
                nc.scalar.activation(out=matt1[:, lo:hi], in_=ml1_ps[:, lo:hi], func=AF.Sigmoid, bias=bias(SB_BA_B, 32))

            mg0 = wp.tile([128, MP], F16, name="mg0")
            mg1 = wp.tile([32, MP], F16, name="mg1")
            mvp = wp.tile([128, 4], F16, name="mvp")   # partial maxes per chunk
            with nc.allow_low_precision(reason="fp16 max-pool rounds values only"):
                for ci, (lo, hi) in enumerate(((0, 512), (512, MP))):
                    nc.vector.scalar_tensor_tensor(out=mg0[:, lo:hi], in0=matt0[:, lo:hi],
                                                   scalar=0.5, in1=mc0[:, lo:hi],
                                                   op0=ALU.add, op1=ALU.mult)
                    nc.vector.tensor_reduce(out=mvp[:, ci:ci + 1], in_=mg0[:, lo:hi],
                                            op=ALU.max, axis=AX.X)
                    nc.vector.scalar_tensor_tensor(out=mg1[:, lo:hi], in0=matt1[:, lo:hi],
                                                   scalar=0.5, in1=mc1[:, lo:hi],
                                                   op0=ALU.add, op1=ALU.mult)
                    nc.vector.tensor_reduce(out=mvp[0:32, 2 + ci:3 + ci], in_=mg1[:, lo:hi],
                                            op=ALU.max, axis=AX.X)
                nc.vector.tensor_reduce(out=pvf[:, 2:3], in_=mvp[:, 0:2], op=ALU.max, axis=AX.X)
                nc.vector.tensor_reduce(out=pvf[0:32, 3:4], in_=mvp[0:32, 2:4], op=ALU.max, axis=AX.X)

            # ================= FC head =================
            def lrelu(name, f_ps, bias_lo, ncols):
                fb = wp.tile([128, ncols], F32, name=name + "_b")
                nc.vector.tensor_tensor(out=fb, in0=f_ps, in1=wsmall[:, bias_lo:bias_lo + ncols], op=ALU.add)
                fs = wp.tile([128, ncols], F32, name=name + "_s")
                nc.vector.tensor_scalar(out=fs, in0=fb, scalar1=0.01, scalar2=None, op0=ALU.mult)
                fo = wp.tile([128, ncols], F16, name=name)
                nc.vector.tensor_tensor(out=fo, in0=fb, in1=fs, op=ALU.max)
                return fo

            # f1: per-column accumulation groups (one 2KB region holds all
            # columns, so groups must not interleave); within a column the
            # two 128-row stationaries go first to reduce PE config flips
            # all K=128 (W1B rows 32:128 and pvf rows 32:128 are zero) so the
            # PE streams without stationary-shape reconfigs; the peptide-side
            # half accumulates in its own psum bank as soon as pv is ready
            # (the MHC max-pool is the critical tail)
            f1p_ps = ps.tile([128, 8], F32, name="f1p_ps", tag="ps")
            for a in range(8):
                nc.tensor.matmul(f1p_ps[:, a:a + 1], wfc[0:128, W1A + a * 128:W1A + a * 128 + 128],
                                 pvf[:, 0:1], start=True, stop=False)
                nc.tensor.matmul(f1p_ps[:, a:a + 1], wfc[0:128, W1B + a * 128:W1B + a * 128 + 128],
                                 pvf[:, 1:2], start=False, stop=True)
            f1_ps = ps.tile([128, 8], F32, name="f1_ps", tag="ps")
            for a in range(8):
                nc.tensor.matmul(f1_ps[:, a:a + 1], wfc[0:128, W1A + 1024 + a * 128:W1A + 1024 + a * 128 + 128],
                                 pvf[:, 2:3], start=True, stop=False)
                nc.tensor.matmul(f1_ps[:, a:a + 1], wfc[0:128, W1B + 1024 + a * 128:W1B + 1024 + a * 128 + 128],
                                 pvf[:, 3:4], start=False, stop=True)
            fbp = wp.tile([128, 8], F32, name="fbp")
            nc.vector.tensor_tensor(out=fbp, in0=f1p_ps, in1=wsmall[:, SB_B1:SB_B1 + 8], op=ALU.add)
            fb1 = wp.tile([128, 8], F32, name="fb1")
            nc.vector.tensor_tensor(out=fb1, in0=f1_ps, in1=fbp, op=ALU.add)
            fs1 = wp.tile([128, 8], F32, name="fs1")
            nc.vector.tensor_scalar(out=fs1, in0=fb1, scalar1=0.01, scalar2=None, op0=ALU.mult)
            f1 = wp.tile([128, 8], F16, name="f1")
            nc.vector.tensor_tensor(out=f1, in0=fb1, in1=fs1, op=ALU.max)

            f2_ps = ps.tile([128, 8], F32, name="f2_ps", tag="ps")
            for a in range(8):
                for jb in range(8):
                    nc.tensor.matmul(f2_ps[:, a:a + 1],
                                     wfc[0:128, W2C + jb * 1024 + a * 128:W2C + jb * 1024 + a * 128 + 128],
                                     f1[:, jb:jb + 1], start=(jb == 0), stop=(jb == 7))
            f2 = lrelu("f2", f2_ps, SB_B2, 8)

            f3_ps = ps.tile([128, 4], F32, name="f3_ps", tag="ps")
            for a in range(4):
                for jb in range(8):
                    nc.tensor.matmul(f3_ps[:, a:a + 1],
                                     wfc[0:128, W3C + jb * 512 + a * 128:W3C + jb * 512 + a * 128 + 128],
                                     f2[:, jb:jb + 1], start=(jb == 0), stop=(jb == 7))
            f3 = lrelu("f3", f3_ps, SB_B3, 4)

            o_ps = ps.tile([2, 1], F32, name="o_ps", tag="ps")
            for c in range(4):
                nc.tensor.matmul(o_ps, wfc[0:128, WOC + 2 * c:WOC + 2 * c + 2], f3[:, c:c + 1],
                                 start=(c == 0), stop=(c == 3))
            o_sb = wp.tile([2, 1], F32, name="o_sb")
            nc.vector.tensor_tensor(out=o_sb, in0=o_ps, in1=wsmall[0:2, SB_BO:SB_BO + 1], op=ALU.add)
            nc.sync.dma_start(out=out_e[:], in_=o_sb)

    _split_excess_waits(nc, max_waits=1)
    return nc


_PROGRAM = None


def _get_program():
    global _PROGRAM
    if _PROGRAM is None:
        _PROGRAM = _build_program()
    return _PROGRAM


def _prep_weights(inp):
    """Host-side packing shared by all cores."""
    import ml_dtypes
    f16 = np.float16
    bf16 = ml_dtypes.bfloat16
    f32 = lambda x: np.asarray(x, dtype=np.float32)
    as_f16bits = lambda a: np.ascontiguousarray(a).view(np.uint16).view(f16)

    def convw(w):  # [co, ci, k] -> [ci, k*co] fp16
        w = np.asarray(w, dtype=np.float32)
        ci = w.shape[1]
        return w.transpose(1, 2, 0).reshape(ci, -1).astype(f16)

    wboot = np.zeros((128, NBOOT), f16)
    def conv1_pairs(w):  # [40, 64, 4] -> two [128, 40] pair stationaries
        w = np.asarray(w, dtype=np.float32)
        out = np.zeros((128, 2 * CONV), np.float32)
        for tp in range(2):
            out[0:DIM, tp * CONV:(tp + 1) * CONV] = w[:, :, 2 * tp].T
            out[DIM:128, tp * CONV:(tp + 1) * CONV] = w[:, :, 2 * tp + 1].T
        return out.astype(f16)
    wboot[:, PW1:PW1 + 2 * CONV] = conv1_pairs(inp['pw1'])
    wboot[:, MW1:MW1 + 2 * CONV] = conv1_pairs(inp['mw1'])
    wboot[0:CONV, PW2:PW2 + K2 * C2] = convw(inp['pw2'])
    wboot[0:CONV, MW2:MW2 + K2 * C2] = convw(inp['mw2'])

    wc3 = np.zeros((128, NC3), f16)
    wc3[0:C2, PW3:PW3 + K3 * C4] = convw(inp['pw3'])
    wc3[0:C2, MW3:MW3 + K3 * C4] = convw(inp['mw3'])

    wpa, wma = f32(inp['Wpa']), f32(inp['Wma'])
    wca = f32(inp['Wa']) / float(LM3)
    wm2 = f32(inp['Wa']) / float(LP3)
    wattn = np.zeros((128, NATTN), f16)
    wattn[0:128, WPA_A:WPA_A + 160] = wpa[0:128].astype(f16)
    wattn[0:32, WPA_B:WPA_B + 160] = wpa[128:160].astype(f16)
    wattn[0:128, WMA_A:WMA_A + 128] = wma[0:128, 0:128].astype(f16)
    wattn[0:128, WMA_A + 128:WMA_A + 256] = np.tile(wma[0:128, 128:160], (1, 4)).astype(f16)
    wattn[0:32, WMA_B:WMA_B + 128] = wma[128:160, 0:128].astype(f16)
    wattn[0:32, WMA_B + 128:WMA_B + 256] = np.tile(wma[128:160, 128:160], (1, 4)).astype(f16)
    wattn[0:128, WCA_A:WCA_A + 160] = as_f16bits(wca[0:128].astype(bf16))
    wattn[0:32, WCA_B:WCA_B + 160] = as_f16bits(wca[128:160].astype(bf16))
    wattn[0:128, WM2_A:WM2_A + 160] = as_f16bits(wm2[0:128].astype(bf16))
    wattn[0:32, WM2_B:WM2_B + 160] = as_f16bits(wm2[128:160].astype(bf16))
    id128 = np.eye(128, dtype=bf16)
    idst = np.tile(np.eye(32, dtype=bf16), (4, 1))
    wattn[0:128, ID128:ID128 + 128] = as_f16bits(id128)
    wattn[0:128, IDST:IDST + 32] = as_f16bits(idst)
    for j in range(4):
        wattn[32 * j:32 * j + 32, WCB0 + j * 128:WCB0 + (j + 1) * 128] = as_f16bits(wca[128:160, 0:128].astype(bf16))
        wattn[32 * j:32 * j + 32, WCB1 + j * 32:WCB1 + (j + 1) * 32] = as_f16bits(wca[128:160, 128:160].astype(bf16))

    w1 = f32(inp['W1'])
    wfc = np.zeros((128, NFC), f16)
    wfc[0:128, W1A:W1A + 2048] = np.concatenate([w1[0:128], w1[160:288]], axis=1).astype(f16)
    wfc[0:32, W1B:W1B + 2048] = np.concatenate([w1[128:160], w1[288:320]], axis=1).astype(f16)

    def fcw(w, nblk):  # [I, J], I = nblk*128 -> [128, nblk*J]
        w = np.asarray(w, dtype=np.float32)
        i, j = w.shape
        return w.reshape(nblk, 128, j).transpose(1, 0, 2).reshape(128, nblk * j).astype(f16)

    wfc[0:128, W2C:W2C + 8192] = fcw(inp['W2'], 8)
    wfc[0:128, W3C:W3C + 4096] = fcw(inp['W3'], 8)
    wfc[0:128, WOC:WOC + 8] = fcw(inp['Wo'], 4)

    wsmall = np.zeros((128, NSMALL), np.float32)
    def bias2(col_a, col_b, b):
        b = f32(b)
        wsmall[0:128, col_a] = b[0:128]
        wsmall[0:32, col_b] = b[128:160]
    wsmall[0:CONV, SB_PB1] = f32(inp['pb1'])
    wsmall[0:C2, SB_PB2] = f32(inp['pb2'])
    bias2(SB_PB3A, SB_PB3B, inp['pb3'])
    wsmall[0:CONV, SB_MB1] = f32(inp['mb1'])
    wsmall[0:C2, SB_MB2] = f32(inp['mb2'])
    bias2(SB_MB3A, SB_MB3B, inp['mb3'])
    bias2(SB_BPA_A, SB_BPA_B, inp['bpa'])
    wsmall[0:128, SB_BMA_A] = f32(inp['bma'])[0:128]
    wsmall[0:128, SB_BMA_R4] = np.tile(f32(inp['bma'])[128:160], 4)
    bias2(SB_BA_A, SB_BA_B, inp['ba'])
    wsmall[0:128, SB_B1:SB_B1 + 8] = f32(inp['b1']).reshape(8, 128).T
    wsmall[0:128, SB_B2:SB_B2 + 8] = f32(inp['b2']).reshape(8, 128).T
    wsmall[0:128, SB_B3:SB_B3 + 4] = f32(inp['b3']).reshape(4, 128).T
    wsmall[0:2, SB_BO] = f32(inp['bo'])

    return {'_wboot': wboot, 'wc3': wc3, 'wattn': wattn, 'wfc': wfc, 'wsmall': wsmall}


def _prep_core(inp, b):
    """Per-core embedding gather: [64, 1100] fp16."""
    pep = np.asarray(inp['peptide'])[b]
    mhc = np.asarray(inp['MHC'])[b]
    pe = np.asarray(inp['pep_emb'], np.float32)[pep].T   # [64, 100]
    me = np.asarray(inp['mhc_emb'], np.float32)[mhc].T   # [64, 1000]
    emb = np.concatenate([pe, me], axis=1)
    out = np.zeros((128, emb.shape[1]), np.float32)
    out[0:64] = emb
    out[64:128, 0:-1] = emb[:, 1:]          # shifted-left copy for tap pairs
    return out.astype(np.float16)


def kernel(**inputs):
    nc = _get_program()
    shared = _prep_weights(inputs)
    wboot = shared.pop('_wboot')
    in_maps = []
    for b in range(B):
        m = dict(shared)
        m['emb'] = np.concatenate([_prep_core(inputs, b), wboot], axis=1)
        in_maps.append(m)
    res = run_bass_kernel_spmd(nc, in_maps, core_ids=list(range(B)))
    return np.stack([np.asarray(res.results[i]['out']).reshape(2) for i in range(B)]).astype(np.float32)
